# revision 1
# baseline (speedup 1.0000x reference)
"""BGConv (GNN message passing) Trainium2 kernel.

Strategy (contribution-ordered, fully host-preprocessed, zero indirect DMA):
  * A "contribution" is an (edge, endpoint) pair: each edge contributes
    sub_feat to node sub and obj_feat to node obj.  Contributions are
    routed to the core owning the destination node and sorted by node.
  * Per core, nodes are grouped into windows (<=128 nodes, <=CH*128
    contributions).  The host packs, per window, a dense record:
      - featsT: per contribution chunk (128 contribs), the gathered pair
        features [feats[sub] | feats[obj]] pre-transposed into the
        [feature-part, contribution-col] layout the PE needs as lhsT.
      - mask:  weighted one-hot matrix m[c, n + 128*half] =
        exp(conf_c - CONST) / denom[node] * WSCALE -- the softmax
        weight, the segment-softmax divide, and the sub/obj split all
        folded in on the host (denominators are host-computable from
        confidence alone).
  * Device, per window (all sequential DMA, no gathers, fp8 DoubleRow
    matmuls throughout): h = relu(featsT^T @ W1) per chunk (whole-chunk
    relus alternate between the Act and DVE engines -- the only two that
    can read PSUM -- and the issue order is software-pipelined two steps
    so the PE never waits on relu), hacc[k, n|n+128] += H^T @ mask with
    H stored kb-interleaved so every DoubleRow operand is a contiguous
    slice (hw silently miscomputes strided-sliced DR operands),
    sp[n,:] = sum_k hacc_sub*W2a + hacc_obj*W2b + I @ selfpart (the
    self/bias term rides the PE as an identity matmul; selfpart =
    rec*(feats + sumw_sub*b2a + sumw_obj*b2b), host-built, f16), then a
    single DVE copy drains sp for the store.  Record/self DMAs are
    batched two windows per transfer; host de-windows the output.
    Steady state is DVE-throughput-bound with zero idle.
  * Softmax max: confidence ~ N(0,1) << CONST=10 so the segment max is
    exactly CONST (asserted on host); w_e = exp(conf_e - 10), self = 1.
  * fp8 (e4m3) is safe here: edge contributions carry ~2-5% of each
    output row (denom ~= 1 + sum w, w ~ exp(-10+conf)); the dominant
    self term is exact f32 on the host.  Measured model error ~2e-4.
"""

import math
import numpy as np
import ml_dtypes

import concourse.bass as bass
import concourse.tile as tile
from concourse import bacc, mybir
from concourse.bass_utils import run_bass_kernel_spmd

# ---------------------------------------------------------------- constants
O_NODES = 50000
N_EDGES = 200000
D = 256
HIDDEN = 512
CONST = 10.0
N_CORES = 8
SHARD = O_NODES // N_CORES          # 6250
P = 128
CH = 8                              # contribution chunks per window
WSCALE = 8192.0                     # keeps fp8 mask weights in normal range
F8 = ml_dtypes.float8_e4m3
FEAT_END = CH * HIDDEN              # feats region end in the record
RECW = CH * HIDDEN + CH * 2 * P     # record cols per window (6144)

_BUILD_CACHE = {}


# ================================================================ host side
def _pack_w(Wm):
    """[4*128, C] -> [128, 4*C] with col-block fb = W[fb*128:(fb+1)*128, :]."""
    C = Wm.shape[1]
    return (
        np.asarray(Wm, dtype=np.float32)
        .reshape(4, P, C).transpose(1, 0, 2).reshape(P, 4 * C)
    )


def _preprocess(object_feats, pairs, confidence, W1, b1, W2, b2):
    object_feats = np.asarray(object_feats, dtype=np.float32)
    pairs = np.asarray(pairs)
    confidence = np.asarray(confidence, dtype=np.float64)
    R = pairs.shape[0]

    conf_max = float(confidence.max())
    assert conf_max < CONST - 1.0, (
        f"kernel assumes segment max == CONST; confidence.max()={conf_max}"
    )

    sub = pairs[:, 0].astype(np.int64)
    obj = pairs[:, 1].astype(np.int64)
    dest = np.concatenate([sub, obj])                       # (2R,)
    eidx = np.concatenate([np.arange(R), np.arange(R)])
    conf2 = np.concatenate([confidence, confidence])
    half2 = np.concatenate([np.zeros(R, np.int64), np.ones(R, np.int64)])

    # softmax weights + per-node denominators (host-exact, f64)
    w_all = np.exp(conf2 - CONST)
    denom = 1.0 + np.bincount(dest, weights=w_all, minlength=O_NODES)
    rec = 1.0 / denom                                       # (O,)
    sumw_sub = np.bincount(sub, weights=np.exp(confidence - CONST),
                           minlength=O_NODES)
    sumw_obj = np.bincount(obj, weights=np.exp(confidence - CONST),
                           minlength=O_NODES)

    order = np.argsort(dest, kind="stable")
    dest_s = dest[order]
    e_s = eidx[order]
    w_s = w_all[order]
    h_s = half2[order]
    core_bounds = np.searchsorted(dest_s, np.arange(N_CORES + 1) * SHARD)

    # ---- window construction per core
    percore = []
    for c in range(N_CORES):
        lo, hi = core_bounds[c], core_bounds[c + 1]
        d_c = (dest_s[lo:hi] - c * SHARD).astype(np.int64)
        deg = np.bincount(d_c, minlength=SHARD)
        wns, wnc, wcs, wcc = [], [], [], []
        n0 = 0
        cpos = 0
        while n0 < SHARD:
            cnt = 0
            contrib = 0
            while n0 + cnt < SHARD and cnt < P:
                dd = deg[n0 + cnt]
                if contrib + dd > CH * P:
                    break
                contrib += dd
                cnt += 1
            assert cnt > 0, "single node exceeds window capacity"
            wns.append(n0); wnc.append(cnt)
            wcs.append(cpos); wcc.append(contrib)
            n0 += cnt
            cpos += contrib
        assert cpos == hi - lo
        percore.append(dict(lo=lo, hi=hi, d=d_c,
                            wns=np.array(wns), wnc=np.array(wnc),
                            wcs=np.array(wcs), wcc=np.array(wcc)))

    W = max(len(pc["wns"]) for pc in percore)
    if W % 2:
        W += 1                                  # window pairs share one DMA
    has_b1 = bool(np.any(np.asarray(b1) != 0.0))

    w1r = _pack_w(W1).astype(F8)                            # [128, 2048] fp8
    w2r = _pack_w(W2).astype(F8)                            # [128, 2048] fp8
    b2a = np.asarray(b2, dtype=np.float64)[:D]
    b2b = np.asarray(b2, dtype=np.float64)[D:]

    in_maps = []
    for c in range(N_CORES):
        pc = percore[c]
        lo, hi = pc["lo"], pc["hi"]
        Nc = hi - lo
        nwin = len(pc["wns"])
        S = W * CH * P                                      # contribution slots

        # slot index for each contribution (window-chunk-row dense layout)
        win_id = np.searchsorted(pc["wcs"], np.arange(Nc), side="right") - 1
        j = np.arange(Nc) - pc["wcs"][win_id]
        slot = win_id * (CH * P) + j

        # gathered pair features -> padded slots
        ec = e_s[lo:hi]
        F = np.zeros((S, 2 * D), dtype=np.float32)
        F[slot, :D] = object_feats[sub[ec]]
        F[slot, D:] = object_feats[obj[ec]]

        # weighted one-hot mask (weight * rec * WSCALE, split by half)
        col = (pc["d"] - pc["wns"][win_id]) + P * h_s[lo:hi]
        mval = (w_s[lo:hi] * rec[dest_s[lo:hi]] * WSCALE).astype(np.float32)
        M = np.zeros((S, 2 * P), dtype=np.float32)
        M[slot, col] = mval

        # record: [W, 128, RECW] = [CH x featsT chunks | CH x mask chunks]
        Wf = (F.reshape(W, CH, P, 4, P)         # [w, cc, c-row, fb, f]
                .transpose(0, 4, 1, 3, 2)       # [w, f, cc, fb, c-row]
                .reshape(W, P, CH * 2 * D))
        Wm = (M.reshape(W, CH, P, 2 * P)        # [w, cc, c-row, col]
                .transpose(0, 2, 1, 3)          # [w, c-row, cc, col]
                .reshape(W, P, CH * 2 * P))
        wrec = np.concatenate([Wf, Wm], axis=2) # [W, 128, RECW]
        # two windows side by side per 128-row block
        wrec = (wrec.reshape(W // 2, 2, P, RECW).transpose(0, 2, 1, 3)
                .reshape(W // 2 * P, 2 * RECW).astype(F8))

        # selfpart, window-pair-dense [W/2*128, 512] f32
        nodes = np.arange(c * SHARD, (c + 1) * SHARD)
        selfn = (rec[nodes, None]
                 * (object_feats[nodes]
                    + sumw_sub[nodes, None] * b2a[None, :]
                    + sumw_obj[nodes, None] * b2b[None, :])).astype(np.float32)
        selfp = np.zeros((W, P, D), dtype=np.float32)
        rowv = []
        nodv = []
        for w in range(nwin):
            ns, cnt = pc["wns"][w], pc["wnc"][w]
            rowv.append(np.arange(w * P, w * P + cnt))
            nodv.append(np.arange(ns, ns + cnt))
        rowv = np.concatenate(rowv)
        nodv = np.concatenate(nodv)
        selfp.reshape(W * P, D)[rowv] = selfn[nodv]
        selfp = (selfp.reshape(W // 2, 2, P, D).transpose(0, 2, 1, 3)
                 .reshape(W // 2 * P, 2 * D).astype(np.float16))

        im = {"wrec": wrec, "selfp": selfp, "w1r": w1r, "w2r": w2r,
              "ident": np.eye(P, dtype=np.float16)}
        if has_b1:
            im["b1rep"] = np.tile(np.asarray(b1, np.float32), (P, 1))
        in_maps.append(im)
    return in_maps, percore, W, has_b1


# ================================================================ device side
def _build_program(W, has_b1):
    dt = mybir.dt
    DR = mybir.MatmulPerfMode.DoubleRow
    NP = CH // 2                                # chunk pairs per window
    nc = bacc.Bacc("TRN2", target_bir_lowering=False, debug=False,
                   num_devices=N_CORES)

    wrec = nc.dram_tensor("wrec", [W // 2 * P, 2 * RECW], dt.float8e4,
                          kind="ExternalInput").ap()
    selfp = nc.dram_tensor("selfp", [W // 2 * P, 2 * D], dt.float16,
                           kind="ExternalInput").ap()
    w1r = nc.dram_tensor("w1r", [P, 4 * HIDDEN], dt.float8e4,
                         kind="ExternalInput").ap()
    w2r = nc.dram_tensor("w2r", [P, 4 * HIDDEN], dt.float8e4,
                         kind="ExternalInput").ap()
    identr = nc.dram_tensor("ident", [P, P], dt.float16,
                            kind="ExternalInput").ap()
    if has_b1:
        b1rep = nc.dram_tensor("b1rep", [P, HIDDEN], dt.float32,
                               kind="ExternalInput").ap()
    outp = nc.dram_tensor("out", [W // 2 * P, 2 * D], dt.float32,
                          kind="ExternalOutput").ap()

    def r2(ap):
        """view cols as [p, 2, half] for DoubleRow"""
        return ap.rearrange("p (two x) -> p two x", two=2)

    with tile.TileContext(nc) as tc:
        with (
            tc.tile_pool(name="const", bufs=1) as const,
            tc.tile_pool(name="wp", bufs=6) as wp,
            tc.tile_pool(name="sfp", bufs=6) as sfp,
            tc.tile_pool(name="Hp", bufs=8) as Hp,
            tc.tile_pool(name="hsp", bufs=4) as hsp,
            tc.tile_pool(name="ep", bufs=6) as ep,
            tc.tile_pool(name="hpsp", bufs=4, space="PSUM") as hpsp,
            tc.tile_pool(name="haccp", bufs=2, space="PSUM") as haccp,
        ):
            w1_s = const.tile([P, 4 * HIDDEN], dt.float8e4)
            nc.sync.dma_start(w1_s[:], w1r[:])
            w2_s = const.tile([P, 4 * HIDDEN], dt.float8e4)
            nc.sync.dma_start(w2_s[:], w2r[:])
            id_s = const.tile([P, P], dt.float16)
            nc.sync.dma_start(id_s[:], identr[:])
            barrier = False
            if has_b1:
                b1_s = const.tile([P, HIDDEN], dt.float32)
                nc.sync.dma_start(b1_s[:], b1rep[:])
            if barrier:
                tc.strict_bb_all_engine_barrier()

            def relu_half(eng, dst, src):
                if eng is nc.scalar:
                    nc.scalar.activation(
                        out=dst, in_=src,
                        func=mybir.ActivationFunctionType.Relu)
                else:
                    eng.tensor_scalar_max(dst, src, 0.0)

            def emit_w1(st, mid=None):
                """W1 matmuls + relus for one chunk pair of a window."""
                wt, pr = st["wt"], st["pr"]
                rbase = st["wo"] * RECW
                Hd = Hp.tile([P, 2 * HIDDEN], dt.float8e4, tag="Hd")
                st["Hd"] = Hd
                for hc in range(2):
                    if hc == 1 and mid is not None:
                        mid()
                    cc = 2 * pr + hc
                    hps = hpsp.tile([P, HIDDEN], dt.float32, tag="hps")
                    for fp in range(2):
                        nc.tensor.matmul(
                            out=hps[:],
                            lhsT=r2(wt[:, rbase + cc * HIDDEN + fp * 2 * P
                                       : rbase + cc * HIDDEN
                                       + (fp + 1) * 2 * P]),
                            rhs=r2(w1_s[:, fp * 2 * HIDDEN
                                        : (fp + 1) * 2 * HIDDEN]),
                            start=(fp == 0),
                            stop=(fp == 1),
                            perf_mode=DR,
                        )
                    # interleaved layout: Hd cols = kb*256 + hc*128 + c so
                    # the Hacc lhsT pairs are contiguous (hw requires it)
                    dst = Hd[:].rearrange("p (kb two c) -> p kb two c",
                                          kb=4, two=2)[:, :, hc : hc + 1, :]
                    if has_b1:
                        hb = Hp.tile([P, HIDDEN], dt.float32, tag="hb")
                        nc.vector.tensor_tensor(
                            out=hb[:], in0=hps[:], in1=b1_s[:],
                            op=mybir.AluOpType.add)
                        nc.scalar.activation(
                            out=dst, in_=hb[:],
                            func=mybir.ActivationFunctionType.Relu)
                    else:
                        # gpsimd cannot read PSUM; alternate Act/DVE
                        eng = (nc.scalar, nc.vector)[hc]
                        relu_half(eng, dst, hps[:])

            def emit_hacc(st, kbs=range(4)):
                wt, pr = st["wt"], st["pr"]
                mbase = st["wo"] * RECW + FEAT_END
                for kb in kbs:
                    nc.tensor.matmul(
                        out=st["hacc"][:, kb * 2 * P : (kb + 1) * 2 * P],
                        lhsT=r2(st["Hd"][:, kb * 2 * P : (kb + 1) * 2 * P]),
                        rhs=r2(wt[:, mbase + pr * 4 * P
                                  : mbase + (pr + 1) * 4 * P]),
                        start=(pr == 0),
                        stop=(pr == NP - 1),
                        perf_mode=DR,
                    )

            def emit_hs(st):
                """hacc psum -> fp8 sbuf, scaled by 1/WSCALE; 4 parallel."""
                hacc = st["hacc"]
                hs = hsp.tile([P, 4 * 2 * P], dt.float8e4, tag="hs")
                st["hs"] = hs
                nc.scalar.activation(
                    out=hs[:, : 6 * P], in_=hacc[:, : 6 * P],
                    func=mybir.ActivationFunctionType.Copy,
                    scale=1.0 / WSCALE)
                nc.vector.tensor_scalar_mul(
                    hs[:, 6 * P :], hacc[:, 6 * P :], 1.0 / WSCALE)

            def emit_w2(st):
                hs = st["hs"]
                wo = st["wo"]
                # sp rides the hps pool ring (same tag/shape): its buffer
                # is not reused until 4 hps allocations later, by which
                # time outt has drained it -- frees a PSUM bank for the
                # 4th hps buffer with normal dependency tracking
                spt = hpsp.tile([P, HIDDEN], dt.float32, name="hps",
                                tag="hps")
                sp = spt[:, :D]
                for kb in range(4):
                    nc.tensor.matmul(
                        out=sp,
                        lhsT=r2(hs[:, kb * 2 * P : (kb + 1) * 2 * P]),
                        rhs=r2(w2_s[:, kb * 4 * P : (kb + 1) * 4 * P]),
                        start=(kb == 0),
                        stop=False,
                        perf_mode=DR,
                    )
                # self-term folded in on the PE: sp += I @ selfpart
                nc.tensor.matmul(
                    out=sp,
                    lhsT=id_s[:],
                    rhs=st["sf"][:, wo * D : (wo + 1) * D],
                    start=False,
                    stop=True,
                )
                outt = ep.tile([P, D], dt.float32, tag="outt")
                nc.vector.tensor_copy(outt[:], sp)
                nc.sync.dma_start(
                    outp[st["wp"] * P : (st["wp"] + 1) * P,
                         wo * D : (wo + 1) * D],
                    outt[:])

            # flat software pipeline over (window, chunk-pair) steps
            steps = []
            shared = {}
            for w in range(W):
                wpair, wo = divmod(w, 2)
                if wo == 0:
                    shared[wpair] = {"wp": wpair}
                for pr in range(NP):
                    steps.append({"w": w, "wo": wo, "pr": pr,
                                  "pair": shared[wpair]})

            win_state = {}
            for i, st in enumerate(steps):
                w, wo, pr, pair = st["w"], st["wo"], st["pr"], st["pair"]
                if wo == 0 and pr == 0:
                    wt = wp.tile([P, 2 * RECW], dt.float8e4, tag="wt")
                    rows = slice(pair["wp"] * P, (pair["wp"] + 1) * P)
                    if pair["wp"] <= 1:
                        # fine-grained early loads so chunk-0 compute starts
                        # as early as possible
                        cuts = (0, HIDDEN, 2 * HIDDEN, FEAT_END, RECW,
                                RECW + FEAT_END, 2 * RECW)
                    else:
                        cuts = (0, RECW, 2 * RECW)
                    for a, b in zip(cuts[:-1], cuts[1:]):
                        nc.sync.dma_start(wt[:, a:b], wrec[rows, a:b])
                    sf = sfp.tile([P, 2 * D], dt.float16, tag="sf")
                    nc.sync.dma_start(
                        sf[:], selfp[pair["wp"] * P : (pair["wp"] + 1) * P, :])
                    pair["wt"], pair["sf"] = wt, sf
                st["wt"] = pair["wt"]
                st["sf"] = pair["sf"]
                st["wp"] = pair["wp"]
                if pr == 0:
                    st["hacc"] = haccp.tile([P, 4 * 2 * P], dt.float32,
                                            name="hacc", tag="hacc")
                    win_state[w] = st["hacc"]
                else:
                    st["hacc"] = win_state[w]

                # hacc trails two steps behind so the PE never waits on
                # relu; its kb-halves are interleaved around the second W1
                # chunk to spread PE work between dependency points
                if i > 1:
                    emit_w1(st, mid=lambda: emit_hacc(steps[i - 2],
                                                      kbs=range(2)))
                    emit_hacc(steps[i - 2], kbs=range(2, 4))
                else:
                    emit_w1(st)
                # epilogue of window w-1, staged after its last hacc
                if pr == 2 and w > 0:
                    emit_hs(steps[i - 3])       # (w-1, NP-1) state
                if pr == 3 and w > 0:
                    emit_w2(steps[i - 4])
            # drain tail
            emit_hacc(steps[-2])
            emit_hacc(steps[-1])
            emit_hs(steps[-1])
            emit_w2(steps[-1])

    nc.compile()
    return nc


# ================================================================ entry point
def kernel(object_feats, pairs, confidence, W1, b1, W2, b2):
    in_maps, percore, W, has_b1 = _preprocess(
        object_feats, pairs, confidence, W1, b1, W2, b2)

    key = (W, has_b1)
    if key not in _BUILD_CACHE:
        _BUILD_CACHE[key] = _build_program(W, has_b1)
    nc = _BUILD_CACHE[key]

    res = run_bass_kernel_spmd(
        nc, in_maps, core_ids=list(range(N_CORES)), trace=False
    )
    out = np.empty((O_NODES, D), dtype=np.float32)
    for c in range(N_CORES):
        ow = (res.results[c]["out"].reshape(W // 2, P, 2, D)
              .transpose(0, 2, 1, 3).reshape(W * P, D))
        pc = percore[c]
        for w in range(len(pc["wns"])):
            ns, cnt = pc["wns"][w], pc["wnc"][w]
            out[c * SHARD + ns : c * SHARD + ns + cnt] = ow[w * P : w * P + cnt]
    return out

